# revision 1
# baseline (speedup 1.0000x reference)
"""Trainium2 Bass kernel for ConditionalHierarchicalCrossEntropyLoss.

Data-parallel: shard y_pred/y_true along batch across 8 NeuronCores;
replicate the tiny per-class table; sum the per-core partials on host.

Per 128-row block on each core:
  1. DMA y_true block [128, 8192] -> SBUF. Two-level argmax: DVE
     per-chunk max reduce [128, 64x128] -> [128, 64]; InstMax+InstMaxIndex
     pick the winning 128-wide chunk; indirect-DMA re-gathers that 512B
     chunk from HBM; InstMaxIndex inside it gives the label
     (first-occurrence at every level, matching jnp.argmax).
  2. gpsimd indirect DMA gathers the per-class record from a host-built
     DRAM table [C, 32]: paths 0..4, node masks [k<len], and
     wm_k = class_w * exp(-0.1*(len-1-k)) * [k < len-1].
  3. gpsimd indirect DMA gathers y_pred[row, path_k], k=0..4, from HBM
     using flat offsets row*8192 + path_k (one offset per partition per
     gather -- the HW DGE constraint). The k=5 entry is always the root
     (class 0) when it is unmasked, so it comes from column 0 of the
     exp'd y_pred tile instead of a 6th gather.
  4. DMA y_pred block -> SBUF; ACT exp-accumulate gives the softmax
     denominator Z per row (no max-subtraction: inputs ~ randn).
  5. Tiny [128, <=8] ops: suffix sums, conditional probs, ln, weighted
     row loss, accumulated across blocks.
Output per core: [128, 1] partials; host: loss = -sum(partials)/B.
"""

import numpy as np

import concourse.bacc as bacc
import concourse.bass as bass
import concourse.tile as tile
from concourse import mybir

N_CORES = 8
B = 8192          # batch
C = 8192          # classes
RPC = B // N_CORES  # rows per core
P = 128           # partitions / rows per block
NBLK = RPC // P   # blocks per core
D = 6             # max tree depth (padded path length)
NCHUNK = 64       # chunks per row for two-level argmax
CW = C // NCHUNK  # chunk width (128)
TW = 32           # table row width (floats)
EPS = 1e-8
DEPTH_PARAM = 0.1

f32 = mybir.dt.float32
u32 = mybir.dt.uint32

Alu = mybir.AluOpType
Act = mybir.ActivationFunctionType


WIN = 585   # tree mode: nodes at depth<=3 live in columns [0, WIN)


def _body(tc, yp_d, yt_d, tab_d, cst_d, cstu_d, out_d, dbg=None, repeats=1,
          tree_mode=False):
    NG = 2 if tree_mode else 5   # number of per-row random gathers
    # Software-pipelined by stage: engines execute in order, so per-block
    # chains with cross-engine round trips would stall every engine for all
    # later blocks. Emitting each stage for all 8 blocks together lets each
    # engine stream 8 homogeneous ops while SWDGE gather latencies amortize.
    nc = tc.nc
    with (
        tc.tile_pool(name="big", bufs=2) as big,
        tc.tile_pool(name="small", bufs=NBLK + 1) as small,
        tc.tile_pool(name="single", bufs=1) as single,
    ):
        cst = single.tile([P, 16], f32)
        nc.sync.dma_start(out=cst[:], in_=cst_d)
        cstu = single.tile([P, 12], u32)
        nc.sync.dma_start(out=cstu[:], in_=cstu_d)
        if tree_mode:
            iota_i = single.tile([P, WIN], mybir.dt.int32)
            nc.gpsimd.iota(iota_i[:], pattern=[[1, WIN]], base=0,
                           channel_multiplier=0)
            iota_f = single.tile([P, WIN], f32)
            nc.vector.tensor_copy(out=iota_f[:], in_=iota_i[:])

        acc = single.tile([P, 1], f32)

        for rep in range(repeats):
            nc.vector.memset(acc[:], 0.0)
            rowbase = [cst[:, 8 + b:9 + b] for b in range(NBLK)]
            st = {k: {} for k in ("cmax", "m8", "cidx", "offc_u", "z", "rz",
                                  "root_e", "chunk", "inner", "inner_f",
                                  "lab_u", "rec", "offu", "g", "eg", "probs",
                                  "sn", "rsn", "cond", "lc", "pl")}

            def stage_a(b):
                # big loads, full-row argmax of y_true, Z of y_pred
                rows = slice(b * P, (b + 1) * P)
                yt = big.tile([P, C], f32, tag="yt", name=f"yt{rep}_{b}")
                nc.sync.dma_start(out=yt[:], in_=yt_d[rows, :])
                yp = big.tile([P, C], f32, tag="yp", name=f"yp{rep}_{b}")
                nc.sync.dma_start(out=yp[:], in_=yp_d[rows, :])
                z = st["z"][b] = small.tile([P, 1], f32, tag="z",
                                            name=f"z{b}")
                nc.scalar.activation(out=yp[:], in_=yp[:], func=Act.Exp,
                                     accum_out=z[:])
                # keep the exp'd low columns (root, and in tree mode all
                # nodes of depth<=3) so yp's big tile can be released early
                wn = WIN if tree_mode else 1
                wc = st["root_e"][b] = small.tile(
                    [P, wn], f32, tag="root_e", name=f"root_e{b}")
                nc.scalar.copy(wc[:], yp[:, 0:wn])
                m8 = st["m8"][b] = small.tile([P, 8], f32, tag="m8",
                                              name=f"m8_{b}")
                nc.vector.max(m8[:], yt[:])
                lab8 = st["lab_u"][b] = small.tile([P, 8], u32, tag="lab_u",
                                                   name=f"lab_u{b}")
                nc.vector.max_index(lab8[:], m8[:], yt[:])

            def stage_b(b):
                # record gather by label
                rec = st["rec"][b] = small.tile([P, TW], f32, tag="rec",
                                                name=f"rec{b}")
                nc.gpsimd.indirect_dma_start(
                    out=rec[:], out_offset=None, in_=tab_d,
                    in_offset=bass.IndirectOffsetOnAxis(
                        ap=st["lab_u"][b][:, 0:1], axis=0),
                )

            def stage_c(b):
                # value offsets; path-logit gathers; 1/Z
                offu = st["offu"][b] = small.tile(
                    [P, NG], u32, tag="offu", name=f"offu{b}")
                nc.vector.tensor_scalar(
                    out=offu[:], in0=st["rec"][b][:, 0:NG],
                    scalar1=rowbase[b], scalar2=None, op0=Alu.add,
                )
                g = st["g"][b] = small.tile([P, NG], f32, tag="g",
                                            name=f"g{b}")
                for k in range(NG):
                    nc.gpsimd.indirect_dma_start(
                        out=g[:, k:k + 1], out_offset=None, in_=yp_d,
                        in_offset=bass.IndirectOffsetOnAxis(
                            ap=offu[:, k:k + 1], axis=1),
                    )
                rz = st["rz"][b] = small.tile([P, 1], f32, tag="rz",
                                              name=f"rz{b}")
                nc.vector.reciprocal(rz[:], st["z"][b][:])

            def stage_e(b):
                # per-row loss tail
                rec, rz = st["rec"][b], st["rz"][b]
                eg = st["eg"][b] = small.tile([P, NG], f32, tag="eg",
                                              name=f"eg{b}")
                nc.scalar.activation(out=eg[:], in_=st["g"][b][:],
                                     func=Act.Exp)
                probs = st["probs"][b] = small.tile(
                    [P, D], f32, tag="probs", name=f"probs{b}")
                wc = st["root_e"][b]
                if tree_mode:
                    # exp'd values for depth<=3 nodes come from the SBUF
                    # window: e_k = sum_j [j == path_k] * exp(x_j)
                    epack = small.tile([P, 4], f32, tag="epack",
                                       name=f"epack{b}")
                    junk = small.tile([P, WIN], f32, tag="junk",
                                      name=f"junk{b}")
                    for i, (kcol, w) in enumerate(((2, WIN), (3, 73),
                                                   (4, 9))):
                        nc.vector.scalar_tensor_tensor(
                            out=junk[:, 0:w], in0=iota_f[:, 0:w],
                            scalar=rec[:, kcol:kcol + 1], in1=wc[:, 0:w],
                            op0=Alu.is_equal, op1=Alu.mult,
                            accum_out=epack[:, i:i + 1],
                        )
                    nc.vector.tensor_copy(out=epack[:, 3:4], in_=wc[:, 0:1])
                    nc.vector.scalar_tensor_tensor(
                        out=probs[:, 0:2], in0=eg[:], scalar=rz[:, 0:1],
                        in1=rec[:, 16:18], op0=Alu.mult, op1=Alu.mult,
                    )
                    nc.vector.scalar_tensor_tensor(
                        out=probs[:, 2:6], in0=epack[:], scalar=rz[:, 0:1],
                        in1=rec[:, 18:22], op0=Alu.mult, op1=Alu.mult,
                    )
                else:
                    nc.vector.scalar_tensor_tensor(
                        out=probs[:, 0:5], in0=eg[:], scalar=rz[:, 0:1],
                        in1=rec[:, 16:21], op0=Alu.mult, op1=Alu.mult,
                    )
                    nc.vector.scalar_tensor_tensor(
                        out=probs[:, 5:6], in0=wc[:], scalar=rz[:, 0:1],
                        in1=rec[:, 21:22], op0=Alu.mult, op1=Alu.mult,
                    )
                for k in range(D - 2, -1, -1):
                    nc.scalar.add(probs[:, k:k + 1], probs[:, k:k + 1],
                                  probs[:, k + 1:k + 2])
                sn = st["sn"][b] = small.tile([P, D - 1], f32, tag="sn",
                                              name=f"sn{b}")
                nc.scalar.activation(out=sn[:], in_=probs[:, 1:6],
                                     func=Act.Identity, bias=cst[:, 7:8])
                rsn = st["rsn"][b] = small.tile([P, D - 1], f32, tag="rsn",
                                                name=f"rsn{b}")
                nc.vector.reciprocal(rsn[:], sn[:])
                cond = st["cond"][b] = small.tile(
                    [P, D - 1], f32, tag="cond", name=f"cond{b}")
                nc.vector.tensor_tensor(out=cond[:], in0=probs[:, 0:5],
                                        in1=rsn[:], op=Alu.mult)
                lc = st["lc"][b] = small.tile([P, D - 1], f32, tag="lc",
                                              name=f"lc{b}")
                nc.scalar.activation(out=lc[:], in_=cond[:], func=Act.Ln,
                                     bias=cst[:, 7:8])
                t2 = small.tile([P, D - 1], f32, tag="t2", name=f"t2_{b}")
                pl = st["pl"][b] = small.tile([P, 1], f32, tag="pl",
                                              name=f"pl{b}")
                nc.vector.scalar_tensor_tensor(
                    out=t2[:], in0=lc[:], scalar=1.0, in1=rec[:, 8:13],
                    op0=Alu.mult, op1=Alu.mult, accum_out=pl[:],
                )
                nc.vector.tensor_tensor(out=acc[:], in0=acc[:], in1=pl[:],
                                        op=Alu.add)

            # software pipeline with block lag so each engine's in-order
            # stream interleaves stages of different blocks
            for s in range(NBLK + 3):
                if s < NBLK:
                    stage_a(s)
                if 0 <= s - 1 < NBLK:
                    stage_b(s - 1)
                if 0 <= s - 2 < NBLK:
                    stage_c(s - 2)
                if 0 <= s - 3 < NBLK:
                    stage_e(s - 3)

            if dbg is not None:
                lab_d, z_d, g_d, pl_d, off_d = dbg
                for b in range(NBLK):
                    rows = slice(b * P, (b + 1) * P)
                    labf = small.tile([P, 1], f32, tag="labf",
                                      name=f"labf{b}")
                    nc.vector.tensor_copy(out=labf[:],
                                          in_=st["lab_u"][b][:, 0:1])
                    nc.sync.dma_start(out=lab_d[rows, :], in_=labf[:])
                    nc.sync.dma_start(out=z_d[rows, :], in_=st["z"][b][:])
                    nc.sync.dma_start(out=g_d[rows, :], in_=st["g"][b][:])
                    nc.sync.dma_start(out=pl_d[rows, :], in_=st["pl"][b][:])
                    nc.sync.dma_start(out=off_d[rows, :],
                                      in_=st["offu"][b][:])

        nc.sync.dma_start(out=out_d, in_=acc[:])


def build_bass(debug_outs=False, repeats=1, tree_mode=False):
    nc = bacc.Bacc("TRN2", target_bir_lowering=False, debug=False,
                   enable_asserts=False)
    yp = nc.dram_tensor("y_pred_s", [RPC, C], f32, kind="ExternalInput")
    yt = nc.dram_tensor("y_true_s", [RPC, C], f32, kind="ExternalInput")
    tab = nc.dram_tensor("table", [C, TW], f32, kind="ExternalInput")
    cst = nc.dram_tensor("consts", [P, 16], f32, kind="ExternalInput")
    cstu = nc.dram_tensor("constsu", [P, 12], u32, kind="ExternalInput")
    out = nc.dram_tensor("partial", [P, 1], f32, kind="ExternalOutput")
    dbg = None
    if debug_outs:
        dbg = (
            nc.dram_tensor("lab_dbg", [RPC, 1], f32, kind="ExternalOutput").ap(),
            nc.dram_tensor("z_dbg", [RPC, 1], f32, kind="ExternalOutput").ap(),
            nc.dram_tensor("g_dbg", [RPC, D - 1], f32,
                           kind="ExternalOutput").ap(),
            nc.dram_tensor("pl_dbg", [RPC, 1], f32, kind="ExternalOutput").ap(),
            nc.dram_tensor("off_dbg", [RPC, D - 1], u32,
                           kind="ExternalOutput").ap(),
        )
    with tile.TileContext(nc) as tc:
        _body(tc, yp.ap(), yt.ap(), tab.ap(), cst.ap(), cstu.ap(), out.ap(),
              dbg, repeats=repeats, tree_mode=tree_mode)
    nc.compile()
    return nc


def make_host_tables(class_w, tree_paths, tree_lens):
    class_w = np.asarray(class_w, np.float64)
    lens = np.asarray(tree_lens, np.float64)
    table = np.zeros((C, TW), np.float32)
    table[:, 0:5] = np.asarray(tree_paths, np.float32)[:, 0:5]
    table[:, 6] = lens.astype(np.float32)
    k5 = np.arange(D - 1, dtype=np.float64)
    h = lens[:, None] - 1.0 - k5[None, :]
    w = np.exp(-DEPTH_PARAM * h.astype(np.float32).astype(np.float64))
    valid = k5[None, :] < (lens[:, None] - 1.0)
    table[:, 8:13] = (class_w[:, None] * w * valid).astype(np.float32)
    k6 = np.arange(D, dtype=np.float64)
    table[:, 16:22] = (k6[None, :] < lens[:, None]).astype(np.float32)

    consts = np.zeros((P, 16), np.float32)
    consts[:, 0:6] = np.arange(D, dtype=np.float32)[None, :]
    consts[:, 6] = 1.0
    consts[:, 7] = EPS
    p_idx = np.arange(P, dtype=np.float32)
    for b in range(NBLK):
        consts[:, 8 + b] = (b * P + p_idx) * C

    constsu = np.zeros((P, 12), np.uint32)
    for b in range(NBLK):
        constsu[:, b] = (b * P + np.arange(P, dtype=np.uint32)) * C
    constsu[:, 8] = CW
    return table, consts, constsu


def make_in_maps(y_pred, y_true, table, consts, constsu):
    y_pred = np.ascontiguousarray(np.asarray(y_pred, np.float32))
    y_true = np.ascontiguousarray(np.asarray(y_true, np.float32))
    in_maps = []
    for c in range(N_CORES):
        in_maps.append({
            "y_pred_s": y_pred[c * RPC:(c + 1) * RPC],
            "y_true_s": y_true[c * RPC:(c + 1) * RPC],
            "table": table,
            "consts": consts,
            "constsu": constsu,
        })
    return in_maps


_NC = {}


def tree_bounds_ok(tree_paths):
    p = np.asarray(tree_paths)
    return bool((p[:, 2].max() < WIN) and (p[:, 3].max() < 73)
                and (p[:, 4].max() < 9))


def kernel(y_pred, y_true, class_w, tree_paths, tree_lens):
    from concourse.bass_utils import run_bass_kernel_spmd
    tm = tree_bounds_ok(tree_paths)
    if tm not in _NC:
        _NC[tm] = build_bass(tree_mode=tm)
    _nc = _NC[tm]
    table, consts, constsu = make_host_tables(class_w, tree_paths, tree_lens)
    in_maps = make_in_maps(y_pred, y_true, table, consts, constsu)
    res = run_bass_kernel_spmd(_nc, in_maps, core_ids=list(range(N_CORES)))
    total = sum(float(r["partial"].sum()) for r in res.results)
    return np.float32(-total / B)


if __name__ == "__main__":
    nc = build_bass()
    print("built OK:", len(nc.m.functions[0].allocations), "allocations")



# revision 9
# speedup vs baseline: 2.7076x; 2.7076x over previous
"""Trainium2 Bass kernel for ConditionalHierarchicalCrossEntropyLoss.

Data-parallel: shard y_pred/y_true along batch across 8 NeuronCores;
replicate the tiny per-class table; sum the per-core partials on host.

Per 128-row block on each core (streamed, 8 blocks per core):
  1. y_true block [128, 8192] -> SBUF on the SP HWDGE queue;
     y_pred block -> SBUF on the Activation HWDGE queue (two independent
     hardware DMA queues so the two 32MB streams never serialize).
  2. DVE full-row top-8 max + max_index give the label (first-occurrence,
     matching jnp.argmax); the y_true tile is freed right after.
  3. ACT exp-accumulate over the y_pred block gives the softmax
     denominator Z per row (no max-subtraction: inputs ~ randn).
  4. gpsimd indirect DMA gathers the per-class record from a host-built
     DRAM table [C, 32]: paths 0..5 (level 5 is always the root), node
     masks [k<len], and wm_k = class_w * exp(-0.1*(len-1-k)) * [k<len-1];
     then 6 single-element gathers pull the raw path logits
     y_pred[row, path_k] straight from HBM (one offset per partition per
     gather -- the HW DGE constraint).
  5. One [128, 6] exp on ACT + tiny gpsimd ops give unnormalized suffix
     sums su; Z cancels in the conditional probability:
     cond = (su_k/Z) / (su_{k+1}/Z + EPS) = su_k / (su_{k+1} + EPS*Z),
     so DVE only runs one tiny reciprocal per block (placed 5 pipeline
     slots behind the max stream so it never stalls it).
  6. The Ln calls for all 8 blocks of a repeat are batched (one
     activation-table swap per repeat) and interleaved into the next
     repeat's stream, so no engine drains at the repeat boundary.
Output per core: [128, 1] partials; host: loss = -sum(partials)/B.
"""

import numpy as np

import concourse.bacc as bacc
import concourse.bass as bass
import concourse.tile as tile
from concourse import mybir

N_CORES = 8
B = 8192          # batch
C = 8192          # classes
RPC = B // N_CORES  # rows per core
P = 128           # partitions / rows per block
NBLK = RPC // P   # blocks per core
D = 6             # max tree depth (padded path length)
TW = 32           # table row width (floats)
EPS = 1e-8
DEPTH_PARAM = 0.1

f32 = mybir.dt.float32
u32 = mybir.dt.uint32

Alu = mybir.AluOpType
Act = mybir.ActivationFunctionType


def _body(tc, yp_d, yt_d, tab_d, cst_d, out_d, repeats=1):
    nc = tc.nc
    TOT = repeats * NBLK
    with (
        tc.tile_pool(name="bigt", bufs=3) as bigt,
        tc.tile_pool(name="bigp", bufs=3) as bigp,
        tc.tile_pool(name="small", bufs=NBLK + 1) as small,
        tc.tile_pool(name="recp", bufs=NBLK + 8) as recp,
        tc.tile_pool(name="junkp", bufs=2) as junkp,
        tc.tile_pool(name="single", bufs=1) as single,
    ):
        cst = single.tile([P, 16], f32)
        nc.sync.dma_start(out=cst[:], in_=cst_d)
        acc = single.tile([P, 1], f32)

        rowbase = [cst[:, 8 + b:9 + b] for b in range(NBLK)]
        st = {k: {} for k in ("yt", "yp", "z", "ez", "m8", "lab",
                              "rec", "graw", "su", "esn", "cond")}

        def stage_a(g):
            # big loads: y_true on SP queue, y_pred on ACT queue
            rows = slice((g % NBLK) * P, (g % NBLK + 1) * P)
            yt = st["yt"][g] = bigt.tile([P, C], f32, tag="yt",
                                         name=f"yt{g}")
            nc.sync.dma_start(out=yt[:], in_=yt_d[rows, :])
            yp = st["yp"][g] = bigp.tile([P, C], f32, tag="yp",
                                         name=f"yp{g}")
            nc.scalar.dma_start(out=yp[:], in_=yp_d[rows, :])

        def stage_b(g):
            # full-row work: exp+Z on ACT, top8 max + argmax on DVE
            yt, yp = st["yt"][g], st["yp"][g]
            z = st["z"][g] = small.tile([P, 1], f32, tag="z", name=f"z{g}")
            nc.scalar.activation(out=yp[:], in_=yp[:], func=Act.Exp,
                                 accum_out=z[:])
            m8 = st["m8"][g] = small.tile([P, 8], f32, tag="m8",
                                          name=f"m8_{g}")
            nc.vector.max(m8[:], yt[:])
            lab = st["lab"][g] = small.tile([P, 8], u32, tag="lab",
                                            name=f"lab{g}")
            nc.vector.max_index(lab[:], m8[:], yt[:])

        def stage_c(g):
            # record gather by label
            rec = st["rec"][g] = recp.tile([P, TW], f32, tag="rec",
                                           name=f"rec{g}")
            nc.gpsimd.indirect_dma_start(
                out=rec[:], out_offset=None, in_=tab_d,
                in_offset=bass.IndirectOffsetOnAxis(
                    ap=st["lab"][g][:, 0:1], axis=0),
            )

        def stage_d(g):
            # flat element offsets; raw path-logit gathers; EPS*Z
            offu = small.tile([P, D], u32, tag="offu", name=f"offu{g}")
            nc.gpsimd.tensor_scalar(
                out=offu[:], in0=st["rec"][g][:, 0:D],
                scalar1=rowbase[g % NBLK], scalar2=None, op0=Alu.add,
            )
            graw = st["graw"][g] = small.tile([P, D], f32, tag="graw",
                                              name=f"graw{g}")
            for k in range(D):
                nc.gpsimd.indirect_dma_start(
                    out=graw[:, k:k + 1], out_offset=None, in_=yp_d,
                    in_offset=bass.IndirectOffsetOnAxis(
                        ap=offu[:, k:k + 1], axis=1),
                )
            ez = st["ez"][g] = small.tile([P, 1], f32, tag="ez",
                                          name=f"ez{g}")
            nc.gpsimd.tensor_scalar(out=ez[:], in0=st["z"][g][:],
                                    scalar1=EPS, scalar2=None,
                                    op0=Alu.mult)

        def stage_e(g):
            # unnormalized suffix sums su and su_next + EPS*Z
            eg = small.tile([P, D], f32, tag="eg", name=f"eg{g}")
            nc.scalar.activation(out=eg[:], in_=st["graw"][g][:],
                                 func=Act.Exp)
            su = st["su"][g] = small.tile([P, D], f32, tag="su",
                                          name=f"su{g}")
            nc.gpsimd.tensor_tensor(out=su[:], in0=eg[:],
                                    in1=st["rec"][g][:, 16:22],
                                    op=Alu.mult)
            for k in range(D - 2, -1, -1):
                nc.gpsimd.tensor_tensor(
                    out=su[:, k:k + 1], in0=su[:, k:k + 1],
                    in1=su[:, k + 1:k + 2], op=Alu.add)
            esn = st["esn"][g] = small.tile([P, D - 1], f32, tag="esn",
                                            name=f"esn{g}")
            nc.gpsimd.tensor_scalar(
                out=esn[:], in0=su[:, 1:D],
                scalar1=st["ez"][g][:, 0:1], scalar2=None, op0=Alu.add)

        def stage_f(g):
            # the only DVE op outside the max stream: one tiny reciprocal
            rsn = small.tile([P, D - 1], f32, tag="rsn", name=f"rsn{g}")
            nc.vector.reciprocal(rsn[:], st["esn"][g][:])
            cond = st["cond"][g] = small.tile([P, D - 1], f32, tag="cond",
                                              name=f"cond{g}")
            nc.gpsimd.tensor_tensor(out=cond[:],
                                    in0=st["su"][g][:, 0:D - 1],
                                    in1=rsn[:], op=Alu.mult)

        def rep_tail(r):
            # batched Ln (one table swap per repeat) + weighted row loss
            g0 = r * NBLK
            lc = {}
            for b in range(NBLK):
                lc[b] = small.tile([P, D - 1], f32, tag="lc",
                                   name=f"lc{g0 + b}")
                nc.scalar.activation(out=lc[b][:],
                                     in_=st["cond"][g0 + b][:],
                                     func=Act.Ln, bias=cst[:, 7:8])
            # weighted row loss on DVE (scalar_tensor_tensor is not
            # supported on the Pool engine)
            nc.vector.memset(acc[:], 0.0)
            for b in range(NBLK):
                t2 = junkp.tile([P, D - 1], f32, tag="t2",
                                name=f"t2_{g0 + b}")
                pl = small.tile([P, 1], f32, tag="pl", name=f"pl{g0 + b}")
                nc.vector.scalar_tensor_tensor(
                    out=t2[:], in0=lc[b][:], scalar=1.0,
                    in1=st["rec"][g0 + b][:, 8:13],
                    op0=Alu.mult, op1=Alu.mult, accum_out=pl[:],
                )
                nc.vector.tensor_tensor(out=acc[:], in0=acc[:], in1=pl[:],
                                        op=Alu.add)

        # software pipeline over all repeats' blocks, oldest stage first:
        # each engine's in-order stream sees ready work before ops that
        # wait on the DMA-bound stream (the yp issue's buffer wait must
        # not block the eg exp behind it on the ACT queue)
        for s in range(TOT + 6):
            if 0 <= s - 5 < TOT:
                stage_f(s - 5)
                if (s - 5) % NBLK == NBLK - 1:
                    rep_tail((s - 5) // NBLK)
            if 0 <= s - 4 < TOT:
                stage_e(s - 4)
            if 0 <= s - 3 < TOT:
                stage_d(s - 3)
            if 0 <= s - 2 < TOT:
                stage_c(s - 2)
            if s < TOT:
                stage_a(s)
            if 0 <= s - 1 < TOT:
                stage_b(s - 1)

        nc.sync.dma_start(out=out_d, in_=acc[:])


def build_bass(debug_outs=False, repeats=1, tree_mode=False):
    nc = bacc.Bacc("TRN2", target_bir_lowering=False, debug=False,
                   enable_asserts=False)
    yp = nc.dram_tensor("y_pred_s", [RPC, C], f32, kind="ExternalInput")
    yt = nc.dram_tensor("y_true_s", [RPC, C], f32, kind="ExternalInput")
    tab = nc.dram_tensor("table", [C, TW], f32, kind="ExternalInput")
    cst = nc.dram_tensor("consts", [P, 16], f32, kind="ExternalInput")
    out = nc.dram_tensor("partial", [P, 1], f32, kind="ExternalOutput")
    with tile.TileContext(nc) as tc:
        _body(tc, yp.ap(), yt.ap(), tab.ap(), cst.ap(), out.ap(),
              repeats=repeats)
    nc.compile()
    return nc


def make_host_tables(class_w, tree_paths, tree_lens):
    class_w = np.asarray(class_w, np.float64)
    lens = np.asarray(tree_lens, np.float64)
    table = np.zeros((C, TW), np.float32)
    table[:, 0:D] = np.asarray(tree_paths, np.float32)[:, 0:D]
    k5 = np.arange(D - 1, dtype=np.float64)
    h = lens[:, None] - 1.0 - k5[None, :]
    w = np.exp(-DEPTH_PARAM * h.astype(np.float32).astype(np.float64))
    valid = k5[None, :] < (lens[:, None] - 1.0)
    table[:, 8:13] = (class_w[:, None] * w * valid).astype(np.float32)
    k6 = np.arange(D, dtype=np.float64)
    table[:, 16:22] = (k6[None, :] < lens[:, None]).astype(np.float32)

    consts = np.zeros((P, 16), np.float32)
    consts[:, 7] = EPS
    p_idx = np.arange(P, dtype=np.float32)
    for b in range(NBLK):
        consts[:, 8 + b] = (b * P + p_idx) * C
    return table, consts, None


def make_in_maps(y_pred, y_true, table, consts, constsu=None):
    y_pred = np.ascontiguousarray(np.asarray(y_pred, np.float32))
    y_true = np.ascontiguousarray(np.asarray(y_true, np.float32))
    in_maps = []
    for c in range(N_CORES):
        in_maps.append({
            "y_pred_s": y_pred[c * RPC:(c + 1) * RPC],
            "y_true_s": y_true[c * RPC:(c + 1) * RPC],
            "table": table,
            "consts": consts,
        })
    return in_maps


_NC = {}


def kernel(y_pred, y_true, class_w, tree_paths, tree_lens):
    from concourse.bass_utils import run_bass_kernel_spmd
    if "k" not in _NC:
        _NC["k"] = build_bass()
    _nc = _NC["k"]
    table, consts, _ = make_host_tables(class_w, tree_paths, tree_lens)
    in_maps = make_in_maps(y_pred, y_true, table, consts)
    res = run_bass_kernel_spmd(_nc, in_maps, core_ids=list(range(N_CORES)))
    total = sum(float(r["partial"].sum()) for r in res.results)
    return np.float32(-total / B)


if __name__ == "__main__":
    nc = build_bass()
    print("built OK:", len(nc.m.functions[0].allocations), "allocations")
